# revision 11
# baseline (speedup 1.0000x reference)
"""DeepseekV3 MoE layer on 8 Trainium2 NeuronCores.

Strategy (expert-parallel, per sharding hint):
- Each core owns 2 of the 16 routed experts. The host routes tokens by top-4
  gate scores (fp32, identical to reference) and ships each core its experts'
  gathered tokens pre-transposed to [H, C] fp16, plus the normalized combine
  weights (host-side gate math, same class of work as the top-k routing).
- The device runs the SwiGLU expert MLPs fp16 (fp32 PSUM), scales outputs by
  the combine weights, and scatter-adds them into a per-core partial-output
  buffer y_acc in DRAM.
- The shared expert is sharded along its intermediate dim (128 of 1024 per
  core), computed weight-stationary so its intermediate lands pre-transposed
  ([i, t]) and the down matmul needs no PE transposes; its dense per-tile
  output initializes y_acc.
- y_acc is reduce-scattered across cores in NHALF token chunks, each fired as
  soon as the scatters touching that chunk complete, so all but the last
  chunk overlap compute. The collective writes fp16 directly into the output;
  the host reassembles and casts (pure unshard, no math).
"""

import os
import sys
import types

sys.path.insert(0, "/opt/trn_rl_repo")

# antenv.axon_hooks shim so trace=True works under axon (profiling only).
if "antenv.axon_hooks" not in sys.modules:
    _hook_holder = [None]
    _hooks_mod = types.ModuleType("antenv.axon_hooks")
    _hooks_mod.set_axon_ntff_profile_hook = lambda h: _hook_holder.__setitem__(0, h)
    _hooks_mod.get_axon_ntff_profile_hook = lambda: _hook_holder[0]
    sys.modules["antenv.axon_hooks"] = _hooks_mod
    try:
        from trn_agent_boot.trn_boot import _ntff_profile_via_ctypes

        _hook_holder[0] = _ntff_profile_via_ctypes("/opt/axon/libaxon_pjrt.so")
    except Exception:
        pass

import numpy as np

import concourse.bass as bass
import concourse.mybir as mybir
from concourse import bacc
from concourse.tile import TileContext, add_dep_helper
from concourse.bass_utils import run_bass_kernel_spmd

N_CORES = 8
T, H, E, I = 2048, 1024, 16, 512
TOPK = 4
SIC = 128  # shared-expert intermediate slice per core (1024 / 8)
EPC = 2  # experts per core
OOB = 1 << 20
NHALF = int(os.environ.get('KERNEL_NHALF', '4'))  # reduce-scatter chunks
TH = T // NHALF
TQ = 4  # dense-write granularity (512-token tiles)

F16 = mybir.dt.float16
F32 = mybir.dt.float32
I32 = mybir.dt.int32
AF = mybir.ActivationFunctionType

_nc_cache = {}
last_exec_time_ns = None


def _build(C_use, C_pad, edges, touch_tq, touch_q):
    NCC = C_pad // 128
    nc = bacc.Bacc(trn_type="TRN2", target_bir_lowering=False, num_devices=N_CORES)

    # ---- I/O ----
    xT16 = nc.dram_tensor("xT16", [H, T], F16, kind="ExternalInput")
    # gathered tokens, transposed: [e, o(h/128), p(h%128), c]
    xgT16 = nc.dram_tensor("xgT16", [EPC, H // 128, 128, C_pad], F16, kind="ExternalInput")
    # gate+up weights packed per expert: [e, 2(g/u), o, p, I]
    wgu16 = nc.dram_tensor("wgu16", [EPC, 2, H // 128, 128, I], F16, kind="ExternalInput")
    # down weights: [e, o(i/128), p(i%128), H]
    wd16 = nc.dram_tensor("wd16", [EPC, I // 128, 128, H], F16, kind="ExternalInput")
    sgsu16 = nc.dram_tensor("sgsu16", [H, 2 * SIC], F16, kind="ExternalInput")
    sd16 = nc.dram_tensor("sd16", [SIC, H], F16, kind="ExternalInput")
    sidx = nc.dram_tensor("sidx", [EPC, NCC, 128], I32, kind="ExternalInput")
    wG = nc.dram_tensor("wG", [EPC, NCC, 128], F32, kind="ExternalInput")

    y_acc = nc.dram_tensor("y_acc", [T, H], F16)
    rows = TH // N_CORES
    rs_b = nc.dram_tensor("rs_b", [NHALF * rows, H], F16)
    y_out = nc.dram_tensor("y_out", [NHALF * rows, H], F16, kind="ExternalOutput")

    SS = 2 * SIC  # 256

    with TileContext(nc) as tc:
        with (
            tc.tile_pool(name="res", bufs=1) as res,
            tc.tile_pool(name="sc", bufs=4) as scp,
            tc.tile_pool(name="yg", bufs=12) as ygp,
            tc.tile_pool(name="ds", bufs=2) as dsp,
            tc.tile_pool(name="ps_a", bufs=4, space="PSUM") as ps_a,
            tc.tile_pool(name="ps_gu", bufs=2, space="PSUM") as ps_gu,
        ):
            # ---- resident tiles ----
            xT_sb = [res.tile([128, H // 128, T // 4], F16, tag=f"xT{q}",
                              name=f"xT_sb{q}")
                     for q in range(4)]
            xgT_sb = res.tile([128, EPC, H // 128, C_pad], F16, tag="xgT")
            wgu_sb = res.tile([128, EPC, 2, H // 128, I], F16, tag="wgu")
            wd_sb = res.tile([128, EPC, I // 128, H], F16, tag="wd")
            sgsu_sb = res.tile([128, H // 128, SS], F16, tag="sgsu")
            sd_sb = res.tile([128, H], F16, tag="sd")
            sidx_sb = res.tile([128, EPC * NCC], I32, tag="sidx")
            wG_sb = res.tile([128, EPC * NCC], F32, tag="wG")
            p_sb = res.tile([128, EPC, I // 128, C_pad], F16, tag="p")
            sp_sb = res.tile([128, T], F16, tag="sp")

            # ---- preload ----
            # scalar queue: shared weights first, then expert gate/up
            nc.scalar.dma_start(sgsu_sb[:], sgsu16.ap().rearrange("(o p) s -> p o s", p=128))
            nc.scalar.dma_start(sd_sb[:], sd16.ap())
            for e in range(EPC):
                nc.scalar.dma_start(
                    wgu_sb[:, e], wgu16.ap()[e].rearrange("k o p i -> p k o i"))
            # sync queue: token activations by quarter + small tensors
            for q in range(4):
                nc.sync.dma_start(
                    xT_sb[q][:],
                    xT16.ap()[:, q * (T // 4):(q + 1) * (T // 4)].rearrange(
                        "(o p) t -> p o t", p=128))
            nc.sync.dma_start(sidx_sb[:], sidx.ap().rearrange("e c p -> p (e c)"))
            nc.sync.dma_start(wG_sb[:], wG.ap().rearrange("e c p -> p (e c)"))
            # gpsimd queue: gathered tokens + down weights (idle until the
            # scatter phase)
            for e in range(EPC):
                nc.gpsimd.dma_start(
                    xgT_sb[:, e], xgT16.ap()[e].rearrange("o p c -> p o c"))
            nc.gpsimd.dma_start(wd_sb[:], wd16.ap().rearrange("e o p h -> p e o h"))

            # zero the pad columns of p (read by down-matmul lhsT chunks)
            if C_pad > C_use:
                nc.vector.memset(p_sb[:, :, :, C_use:C_pad], 0)

            # gate/up token segments (PSUM free-dim limit 512)
            segs = []
            s0 = 0
            while s0 < C_use:
                s1 = min(s0 + 512, C_use)
                segs.append((s0, s1))
                s0 = s1

            QT = T // 4  # tokens per xT quarter / dense tile

            # ---- shared expert: weight-stationary gate/up for one 512-token
            # quarter; sp lands pre-transposed [i, t] ----
            def emit_su(tq):
                ps_ic = []
                for ic in range(2):
                    psu = ps_a.tile([128, QT], F32, tag="psa")
                    for ho in range(H // 128):
                        nc.tensor.matmul(
                            psu[:],
                            lhsT=sgsu_sb[:, ho, ic * 128:(ic + 1) * 128],
                            rhs=xT_sb[tq][:, ho, :],
                            start=(ho == 0),
                            stop=(ho == H // 128 - 1),
                        )
                    ps_ic.append(psu)
                sg_t = scp.tile([128, QT], F16, tag="sg")
                nc.scalar.activation(sg_t[:], ps_ic[0][:], AF.Silu)
                nc.vector.tensor_tensor(
                    out=sp_sb[:, tq * QT:(tq + 1) * QT], in0=sg_t[:], in1=ps_ic[1][:],
                    op=mybir.AluOpType.mult,
                )

            # ---- dense shared-expert partial for one 512-token tile ->
            # initializes y_acc rows [tq*512, (tq+1)*512) ----
            dense_wr = [None] * TQ

            def emit_dense(tq):
                ys = dsp.tile([128, 4, H], F16, tag="ys")
                for tc4 in range(4):
                    t0 = tq * QT + tc4 * 128
                    for hf in range(2):
                        pso = ps_a.tile([128, 512], F32, tag="psa")
                        nc.tensor.matmul(
                            pso[:],
                            lhsT=sp_sb[:, t0:t0 + 128],
                            rhs=sd_sb[:, hf * 512:(hf + 1) * 512],
                            start=True,
                            stop=True,
                        )
                        nc.vector.tensor_copy(
                            ys[:, tc4, hf * 512:(hf + 1) * 512], pso[:])
                dense_wr[tq] = nc.sync.dma_start(
                    y_acc.ap()[tq * QT:(tq + 1) * QT, :].rearrange(
                        "(tc p) h -> p tc h", p=128),
                    ys[:],
                )

            # ---- routed experts: g/u -> p = silu(g)*u for one token segment ----
            def emit_gu(e, a, b):
                for it in range(I // 128):
                    pg_full = ps_gu.tile([128, 512], F32, tag="pg")
                    pg = pg_full[:, :b - a]
                    pu_full = ps_gu.tile([128, 512], F32, tag="pu")
                    pu = pu_full[:, :b - a]
                    for ho in range(H // 128):
                        nc.tensor.matmul(
                            pg[:],
                            lhsT=wgu_sb[:, e, 0, ho, it * 128:(it + 1) * 128],
                            rhs=xgT_sb[:, e, ho, a:b],
                            start=(ho == 0),
                            stop=(ho == H // 128 - 1),
                        )
                        nc.tensor.matmul(
                            pu[:],
                            lhsT=wgu_sb[:, e, 1, ho, it * 128:(it + 1) * 128],
                            rhs=xgT_sb[:, e, ho, a:b],
                            start=(ho == 0),
                            stop=(ho == H // 128 - 1),
                        )
                    sg2_full = scp.tile([128, 512], F16, tag="sg2")
                    sg2 = sg2_full[:, :b - a]
                    nc.scalar.activation(sg2[:], pg[:], AF.Silu)
                    nc.vector.tensor_tensor(
                        out=p_sb[:, e, it, a:b], in0=sg2[:], in1=pu[:],
                        op=mybir.AluOpType.mult,
                    )

            # ---- routed expert down matmul + combine-weight scale ----
            yg_tiles = {}
            scat_insts = {}
            rs_insts = [None] * NHALF

            def emit_down(e, cc):
                j = e * NCC + cc
                yg = ygp.tile([128, H], F16, tag="ygtile")
                for hf in range(2):
                    py = ps_a.tile([128, 512], F32, tag="psa")
                    for it in range(I // 128):
                        nc.tensor.matmul(
                            py[:],
                            lhsT=p_sb[:, e, it, cc * 128:(cc + 1) * 128],
                            rhs=wd_sb[:, e, it, hf * 512:(hf + 1) * 512],
                            start=(it == 0),
                            stop=(it == I // 128 - 1),
                        )
                    nc.vector.tensor_scalar_mul(
                        yg[:, hf * 512:(hf + 1) * 512], py[:], wG_sb[:, j:j + 1])
                yg_tiles[(e, cc)] = yg

            def emit_scatter(e, cc):
                j = e * NCC + cc
                sc = nc.gpsimd.indirect_dma_start(
                    out=y_acc[:],
                    out_offset=bass.IndirectOffsetOnAxis(
                        ap=sidx_sb[:, j:j + 1], axis=0),
                    in_=yg_tiles[(e, cc)][:],
                    in_offset=None,
                    bounds_check=T - 1,
                    oob_is_err=False,
                    compute_op=mybir.AluOpType.add,
                )
                for tq in touch_tq.get((e, cc), ()):
                    add_dep_helper(sc.ins, dense_wr[tq].ins,
                                   reason="scatter after dense init")
                for (i0, jj) in edges:
                    other = None
                    if e == 1 and jj == cc:
                        other = (0, i0)
                    elif e == 0 and i0 == cc:
                        other = (1, jj)
                    if other is not None and other in scat_insts:
                        add_dep_helper(sc.ins, scat_insts[other].ins,
                                       reason="serialize colliding scatters")
                scat_insts[(e, cc)] = sc

            def emit_rs(h):
                cc_inst = nc.gpsimd.collective_compute(
                    "ReduceScatter",
                    mybir.AluOpType.add,
                    replica_groups=[list(range(N_CORES))],
                    ins=[y_acc.ap()[h * TH:(h + 1) * TH, :].opt()],
                    outs=[rs_b.ap()[h * rows:(h + 1) * rows, :].opt()],
                )
                for k in touch_q.get(h, ()):
                    if k in scat_insts:
                        add_dep_helper(cc_inst.ins, scat_insts[k].ins,
                                       reason="rs after scatters")
                for tq in range(TQ):
                    if (tq * QT < (h + 1) * TH) and ((tq + 1) * QT > h * TH):
                        add_dep_helper(cc_inst.ins, dense_wr[tq].ins,
                                       reason="rs after dense init")
                rs_insts[h] = cc_inst
                out_wr = nc.sync.dma_start(
                    y_out.ap()[h * rows:(h + 1) * rows, :],
                    rs_b.ap()[h * rows:(h + 1) * rows, :],
                )
                add_dep_helper(out_wr.ins, cc_inst.ins, reason="copy rs out")

            def maybe_rs():
                for h in range(NHALF):
                    if rs_insts[h] is None:
                        if all(k in scat_insts for k in touch_q.get(h, ())):
                            emit_rs(h)
                        else:
                            break  # keep cc-stream order h ascending

            # ---- emission order: keep the PE continuously fed, start the
            # scatter/RS pipeline as early as possible ----
            emit_su(0)
            emit_su(1)
            emit_dense(0)
            seg0 = segs[0]
            emit_gu(0, *seg0)
            emit_su(2)
            emit_dense(1)
            emit_gu(1, *seg0)
            emit_su(3)
            emit_dense(2)
            emit_dense(3)

            cc_seg0 = [cc for cc in range(NCC) if (cc + 1) * 128 <= seg0[1]]
            cc_rest = [cc for cc in range(NCC) if cc not in cc_seg0]

            def down_block(cc_list):
                for cc in cc_list:
                    emit_down(0, cc)
                    emit_down(1, cc)
                    emit_scatter(0, cc)
                    if cc > 0:
                        emit_scatter(1, cc - 1)
                    maybe_rs()

            down_block(cc_seg0)
            for (a, b) in segs[1:]:
                emit_gu(0, a, b)
                emit_gu(1, a, b)
            down_block(cc_rest)
            emit_scatter(1, NCC - 1)
            maybe_rs()
            assert all(r is not None for r in rs_insts)

    nc.compile()
    return nc


def _get_nc(key):
    if key not in _nc_cache:
        _nc_cache[key] = _build(*key)
    return _nc_cache[key]


def kernel(hidden_states, gate_w, expert_gate, expert_up, expert_down,
           shared_gate, shared_up, shared_down):
    global last_exec_time_ns
    B, S, Hh = hidden_states.shape
    x = np.asarray(hidden_states, np.float32).reshape(-1, Hh)

    # ---- host-side routing (the all-to-all dispatch, done as sharding) ----
    gw = np.asarray(gate_w, np.float32)
    logits = x @ gw.T
    scores = 1.0 / (1.0 + np.exp(-logits))
    # top-4 per token; stable sort matches jax.lax.top_k tie semantics
    order = np.argsort(-scores, axis=1, kind="stable")[:, :TOPK]
    topk_w = np.take_along_axis(scores, order, axis=1)
    topk_w = topk_w / (topk_w.sum(-1, keepdims=True) + 1e-20)
    comb = np.zeros((T, E), np.float32)
    np.add.at(comb, (np.arange(T)[:, None], order), topk_w)

    sel = np.zeros((T, E), dtype=bool)
    sel[np.arange(T)[:, None], order] = True
    counts = sel.sum(0)
    C_use = int(max(64, -(-int(counts.max()) // 64) * 64))
    C_use = min(C_use, T)
    C_pad = -(-C_use // 128) * 128
    NCC = C_pad // 128

    gidx_all = np.zeros((E, C_pad), np.int32)
    sidx_all = np.full((E, C_pad), OOB, np.int32)
    for e in range(E):
        lst = np.nonzero(sel[:, e])[0].astype(np.int32)
        gidx_all[e, :len(lst)] = lst
        sidx_all[e, :len(lst)] = lst

    # ---- cast / pack per-core inputs ----
    x16 = x.astype(np.float16)
    xT16 = np.ascontiguousarray(x16.T)
    eg = np.asarray(expert_gate, np.float32).astype(np.float16)
    eu = np.asarray(expert_up, np.float32).astype(np.float16)
    ed = np.asarray(expert_down, np.float32).astype(np.float16)
    sg = np.asarray(shared_gate, np.float32).astype(np.float16)
    su = np.asarray(shared_up, np.float32).astype(np.float16)
    sd = np.asarray(shared_down, np.float32).astype(np.float16)

    in_maps = []
    for c in range(N_CORES):
        ex = [EPC * c + k for k in range(EPC)]
        # gathered + transposed tokens per local expert: [e, o, p, C_pad]
        xgT = np.stack([
            np.ascontiguousarray(x16[gidx_all[e]].T.reshape(H // 128, 128, C_pad))
            for e in ex
        ])
        wgu = np.stack([
            np.stack([eg[e].reshape(H // 128, 128, I),
                      eu[e].reshape(H // 128, 128, I)])
            for e in ex
        ])
        wdp = np.stack([ed[e].reshape(I // 128, 128, H) for e in ex])
        wGc = np.stack([
            comb[gidx_all[e], e].astype(np.float32).reshape(NCC, 128)
            for e in ex
        ])
        # zero pad-row weights (pad rows also scatter to OOB and are dropped)
        for k, e in enumerate(ex):
            nreal = int(counts[e])
            wGc[k].reshape(-1)[nreal:] = 0.0
        in_maps.append({
            "xT16": xT16,
            "xgT16": xgT,
            "wgu16": np.ascontiguousarray(wgu),
            "wd16": np.ascontiguousarray(wdp),
            "sgsu16": np.ascontiguousarray(
                np.concatenate([sg[:, c * SIC:(c + 1) * SIC],
                                su[:, c * SIC:(c + 1) * SIC]], axis=1)),
            "sd16": np.ascontiguousarray(sd[c * SIC:(c + 1) * SIC, :]),
            "sidx": np.ascontiguousarray(sidx_all[ex].reshape(EPC, NCC, 128)),
            "wG": np.ascontiguousarray(wGc),
        })

    # collision edges between the two local experts' scatter chunks, plus the
    # dense tiles / RS chunks each (expert, chunk) scatter touches (union
    # across cores — SPMD shares one program)
    edge_set = set()
    touch_tq = {}
    touch_q = {}
    QT = T // 4
    for c in range(N_CORES):
        pair = [EPC * c, EPC * c + 1]
        rng = {}
        for k, e in enumerate(pair):
            for i in range(NCC):
                r = sidx_all[e, i * 128:(i + 1) * 128]
                r = r[r < OOB]
                if len(r):
                    lo, hi = int(r.min()), int(r.max())
                    rng[(k, i)] = (lo, hi)
                    for tq in range(lo // QT, hi // QT + 1):
                        touch_tq.setdefault((k, i), set()).add(tq)
                    for h in range(lo // TH, hi // TH + 1):
                        touch_q.setdefault(h, set()).add((k, i))
        for i in range(NCC):
            for jj in range(NCC):
                a = rng.get((0, i))
                b = rng.get((1, jj))
                if a and b and a[0] <= b[1] and b[0] <= a[1]:
                    edge_set.add((i, jj))
    edges = tuple(sorted(edge_set))
    touch_tq_t = tuple(sorted((k, tuple(sorted(v))) for k, v in touch_tq.items()))
    touch_q_t = tuple(sorted((h, tuple(sorted(v))) for h, v in touch_q.items()))

    key = (C_use, C_pad, edges, touch_tq_t, touch_q_t, NHALF)
    if key not in _nc_cache:
        _nc_cache[key] = _build(
            C_use, C_pad, edges,
            {k: v for k, v in touch_tq_t}, {h: v for h, v in touch_q_t})
    nc = _nc_cache[key]
    trace = bool(int(os.environ.get("KERNEL_TRACE", "0")))
    res = run_bass_kernel_spmd(
        nc, in_maps, core_ids=list(range(N_CORES)), trace=trace
    )
    last_exec_time_ns = res.exec_time_ns

    # reassemble: RS chunk h gives core c rows [h*TH + c*rows : +rows]
    rows = TH // N_CORES
    out = np.empty((T, Hh), np.float32)
    for c in range(N_CORES):
        yo = res.results[c]["y_out"]
        for h in range(NHALF):
            out[h * TH + c * rows:h * TH + (c + 1) * rows] = yo[h * rows:(h + 1) * rows]
    return out.reshape(B, S, Hh).astype(np.float32)


# revision 16
# speedup vs baseline: 1.1342x; 1.1342x over previous
"""DeepseekV3 MoE layer on 8 Trainium2 NeuronCores.

Strategy (expert-parallel, per sharding hint):
- Each core owns 2 of the 16 routed experts. The host routes tokens by top-4
  gate scores (fp32, identical to reference) and ships each core its experts'
  gathered tokens pre-transposed to [H, C] fp16, plus the normalized combine
  weights (host-side gate math, same class of work as the top-k routing).
- The device runs the SwiGLU expert MLPs fp16 (fp32 PSUM), scales outputs by
  the combine weights, and scatter-adds them into a per-core partial-output
  buffer y_acc in DRAM.
- The shared expert is sharded along its intermediate dim (128 of 1024 per
  core), computed weight-stationary so its intermediate lands pre-transposed
  ([i, t]) and the down matmul needs no PE transposes; its dense per-tile
  output initializes y_acc.
- y_acc is reduce-scattered across cores in NHALF token chunks, each fired as
  soon as the scatters touching that chunk complete, so all but the last
  chunk overlap compute. The collective writes fp16 directly into the output;
  the host reassembles and casts (pure unshard, no math).
"""

import os
import sys
import types

sys.path.insert(0, "/opt/trn_rl_repo")

# antenv.axon_hooks shim so trace=True works under axon (profiling only).
if "antenv.axon_hooks" not in sys.modules:
    _hook_holder = [None]
    _hooks_mod = types.ModuleType("antenv.axon_hooks")
    _hooks_mod.set_axon_ntff_profile_hook = lambda h: _hook_holder.__setitem__(0, h)
    _hooks_mod.get_axon_ntff_profile_hook = lambda: _hook_holder[0]
    sys.modules["antenv.axon_hooks"] = _hooks_mod
    try:
        from trn_agent_boot.trn_boot import _ntff_profile_via_ctypes

        _hook_holder[0] = _ntff_profile_via_ctypes("/opt/axon/libaxon_pjrt.so")
    except Exception:
        pass

import numpy as np

import concourse.bass as bass
import concourse.mybir as mybir
from concourse import bacc
from concourse.tile import TileContext, add_dep_helper
from concourse.bass_utils import run_bass_kernel_spmd

N_CORES = 8
T, H, E, I = 2048, 1024, 16, 512
TOPK = 4
SIC = 128  # shared-expert intermediate slice per core (1024 / 8)
EPC = 2  # experts per core
OOB = 1 << 20
NHALF = int(os.environ.get('KERNEL_NHALF', '2'))  # reduce-scatter chunks
TH = T // NHALF
TQ = 4  # dense-write granularity (512-token tiles)

F16 = mybir.dt.float16
F32 = mybir.dt.float32
I32 = mybir.dt.int32
AF = mybir.ActivationFunctionType

_nc_cache = {}
last_exec_time_ns = None


def _build(C_use, C_pad, edges, touch_tq, touch_q):
    NCC = C_pad // 128
    nc = bacc.Bacc(trn_type="TRN2", target_bir_lowering=False, num_devices=N_CORES)

    # ---- I/O ----
    xT16 = nc.dram_tensor("xT16", [H, T], F16, kind="ExternalInput")
    # gathered tokens, transposed: [e, o(h/128), p(h%128), c]
    xgT16 = nc.dram_tensor("xgT16", [EPC, H // 128, 128, C_pad], F16, kind="ExternalInput")
    # gate+up weights packed per expert: [e, 2(g/u), o, p, I]
    wgu16 = nc.dram_tensor("wgu16", [EPC, 2, H // 128, 128, I], F16, kind="ExternalInput")
    # down weights: [e, o(i/128), p(i%128), H]
    wd16 = nc.dram_tensor("wd16", [EPC, I // 128, 128, H], F16, kind="ExternalInput")
    sgsu16 = nc.dram_tensor("sgsu16", [H, 2 * SIC], F16, kind="ExternalInput")
    sd16 = nc.dram_tensor("sd16", [SIC, H], F16, kind="ExternalInput")
    sidx = nc.dram_tensor("sidx", [EPC, NCC, 128], I32, kind="ExternalInput")
    wG = nc.dram_tensor("wG", [EPC, NCC, 128], F32, kind="ExternalInput")

    y_acc = nc.dram_tensor("y_acc", [T, H], F16)
    rows = TH // N_CORES
    rs_b = nc.dram_tensor("rs_b", [NHALF * rows, H], F16)
    y_out = nc.dram_tensor("y_out", [NHALF * rows, H], F16, kind="ExternalOutput")

    SS = 2 * SIC  # 256

    with TileContext(nc) as tc:
        with (
            tc.tile_pool(name="res", bufs=1) as res,
            tc.tile_pool(name="sc", bufs=4) as scp,
            tc.tile_pool(name="yg", bufs=12) as ygp,
            tc.tile_pool(name="ds", bufs=2) as dsp,
            tc.tile_pool(name="ps_a", bufs=4, space="PSUM") as ps_a,
            tc.tile_pool(name="ps_gu", bufs=2, space="PSUM") as ps_gu,
        ):
            # ---- resident tiles ----
            xT_sb = [res.tile([128, H // 128, T // 4], F16, tag=f"xT{q}",
                              name=f"xT_sb{q}")
                     for q in range(4)]
            xgT_sb = res.tile([128, EPC, H // 128, C_pad], F16, tag="xgT")
            wgu_sb = res.tile([128, EPC, 2, H // 128, I], F16, tag="wgu")
            wd_sb = res.tile([128, EPC, I // 128, H], F16, tag="wd")
            sgsu_sb = res.tile([128, H // 128, SS], F16, tag="sgsu")
            sd_sb = res.tile([128, H], F16, tag="sd")
            sidx_sb = res.tile([128, EPC * NCC], I32, tag="sidx")
            wG_sb = res.tile([128, EPC * NCC], F32, tag="wG")
            p_sb = res.tile([128, EPC, I // 128, C_pad], F16, tag="p")
            sp_sb = res.tile([128, T], F16, tag="sp")

            # ---- preload (per-queue DMA BW is ~100GB/s; balance the three
            # queues and order each by first-use time) ----
            # sync queue: shared-expert inputs + token activations by quarter
            nc.sync.dma_start(sd_sb[:], sd16.ap())
            for q in range(4):
                nc.sync.dma_start(
                    xT_sb[q][:],
                    xT16.ap()[:, q * (T // 4):(q + 1) * (T // 4)].rearrange(
                        "(o p) t -> p o t", p=128))
            nc.sync.dma_start(sidx_sb[:], sidx.ap().rearrange("e c p -> p (e c)"))
            nc.sync.dma_start(wG_sb[:], wG.ap().rearrange("e c p -> p (e c)"))
            # scalar queue: shared gate/up, expert-0 gate/up, down weights
            nc.scalar.dma_start(sgsu_sb[:], sgsu16.ap().rearrange("(o p) s -> p o s", p=128))
            nc.scalar.dma_start(
                wgu_sb[:, 0], wgu16.ap()[0].rearrange("k o p i -> p k o i"))
            for e in range(EPC):
                nc.scalar.dma_start(
                    wd_sb[:, e], wd16.ap()[e].rearrange("o p h -> p o h"))
            # gpsimd queue: gathered tokens + expert-1 gate/up (must drain
            # before the scatter phase starts on this queue)
            nc.gpsimd.dma_start(
                xgT_sb[:, 0], xgT16.ap()[0].rearrange("o p c -> p o c"))
            nc.gpsimd.dma_start(
                wgu_sb[:, 1], wgu16.ap()[1].rearrange("k o p i -> p k o i"))
            nc.gpsimd.dma_start(
                xgT_sb[:, 1], xgT16.ap()[1].rearrange("o p c -> p o c"))

            # zero the pad columns of p (read by down-matmul lhsT chunks)
            if C_pad > C_use:
                nc.vector.memset(p_sb[:, :, :, C_use:C_pad], 0)

            # gate/up token blocks of 256 (ldweights still overlaps matmuls at
            # 256-col moving dim; smaller blocks let downs/scatters start
            # earlier so the reduce-scatter chunks overlap compute)
            segs = []
            s0 = 0
            while s0 < C_use:
                s1 = min(s0 + 256, C_use)
                segs.append((s0, s1))
                s0 = s1

            QT = T // 4  # tokens per xT quarter / dense tile

            # ---- shared expert: weight-stationary gate/up for one 512-token
            # quarter; sp lands pre-transposed [i, t] ----
            def emit_su(tq):
                ps_ic = []
                for ic in range(2):
                    psu = ps_a.tile([128, QT], F32, tag="psa")
                    for ho in range(H // 128):
                        nc.tensor.matmul(
                            psu[:],
                            lhsT=sgsu_sb[:, ho, ic * 128:(ic + 1) * 128],
                            rhs=xT_sb[tq][:, ho, :],
                            start=(ho == 0),
                            stop=(ho == H // 128 - 1),
                        )
                    ps_ic.append(psu)
                sg_t = scp.tile([128, QT], F16, tag="sg")
                nc.scalar.activation(sg_t[:], ps_ic[0][:], AF.Silu)
                nc.vector.tensor_tensor(
                    out=sp_sb[:, tq * QT:(tq + 1) * QT], in0=sg_t[:], in1=ps_ic[1][:],
                    op=mybir.AluOpType.mult,
                )

            # ---- dense shared-expert partial for one 512-token tile ->
            # initializes y_acc rows [tq*512, (tq+1)*512) ----
            dense_wr = [None] * TQ

            def emit_dense(tq):
                ys = dsp.tile([128, 4, H], F16, tag="ys")
                for tc4 in range(4):
                    t0 = tq * QT + tc4 * 128
                    for hf in range(2):
                        pso = ps_a.tile([128, 512], F32, tag="psa")
                        nc.tensor.matmul(
                            pso[:],
                            lhsT=sp_sb[:, t0:t0 + 128],
                            rhs=sd_sb[:, hf * 512:(hf + 1) * 512],
                            start=True,
                            stop=True,
                        )
                        nc.vector.tensor_copy(
                            ys[:, tc4, hf * 512:(hf + 1) * 512], pso[:])
                dense_wr[tq] = nc.scalar.dma_start(
                    y_acc.ap()[tq * QT:(tq + 1) * QT, :].rearrange(
                        "(tc p) h -> p tc h", p=128),
                    ys[:],
                )

            # ---- routed experts: g/u -> p = silu(g)*u for one token segment ----
            def emit_gu(e, a, b):
                for it in range(I // 128):
                    pg_full = ps_gu.tile([128, 512], F32, tag="pg")
                    pg = pg_full[:, :b - a]
                    pu_full = ps_gu.tile([128, 512], F32, tag="pu")
                    pu = pu_full[:, :b - a]
                    for ho in range(H // 128):
                        nc.tensor.matmul(
                            pg[:],
                            lhsT=wgu_sb[:, e, 0, ho, it * 128:(it + 1) * 128],
                            rhs=xgT_sb[:, e, ho, a:b],
                            start=(ho == 0),
                            stop=(ho == H // 128 - 1),
                        )
                        nc.tensor.matmul(
                            pu[:],
                            lhsT=wgu_sb[:, e, 1, ho, it * 128:(it + 1) * 128],
                            rhs=xgT_sb[:, e, ho, a:b],
                            start=(ho == 0),
                            stop=(ho == H // 128 - 1),
                        )
                    sg2_full = scp.tile([128, 512], F16, tag="sg2")
                    sg2 = sg2_full[:, :b - a]
                    nc.scalar.activation(sg2[:], pg[:], AF.Silu)
                    nc.vector.tensor_tensor(
                        out=p_sb[:, e, it, a:b], in0=sg2[:], in1=pu[:],
                        op=mybir.AluOpType.mult,
                    )

            # ---- routed expert down matmul + combine-weight scale ----
            yg_tiles = {}
            scat_insts = {}
            rs_insts = [None] * NHALF

            def emit_down(e, cc):
                j = e * NCC + cc
                yg = ygp.tile([128, H], F16, tag="ygtile")
                for hf in range(2):
                    py = ps_a.tile([128, 512], F32, tag="psa")
                    for it in range(I // 128):
                        nc.tensor.matmul(
                            py[:],
                            lhsT=p_sb[:, e, it, cc * 128:(cc + 1) * 128],
                            rhs=wd_sb[:, e, it, hf * 512:(hf + 1) * 512],
                            start=(it == 0),
                            stop=(it == I // 128 - 1),
                        )
                    nc.vector.tensor_scalar_mul(
                        yg[:, hf * 512:(hf + 1) * 512], py[:], wG_sb[:, j:j + 1])
                yg_tiles[(e, cc)] = yg

            def emit_scatter(e, cc):
                j = e * NCC + cc
                sc = nc.gpsimd.indirect_dma_start(
                    out=y_acc[:],
                    out_offset=bass.IndirectOffsetOnAxis(
                        ap=sidx_sb[:, j:j + 1], axis=0),
                    in_=yg_tiles[(e, cc)][:],
                    in_offset=None,
                    bounds_check=T - 1,
                    oob_is_err=False,
                    compute_op=mybir.AluOpType.add,
                )
                for tq in touch_tq.get((e, cc), ()):
                    add_dep_helper(sc.ins, dense_wr[tq].ins,
                                   reason="scatter after dense init")
                for (i0, jj) in edges:
                    other = None
                    if e == 1 and jj == cc:
                        other = (0, i0)
                    elif e == 0 and i0 == cc:
                        other = (1, jj)
                    if other is not None and other in scat_insts:
                        add_dep_helper(sc.ins, scat_insts[other].ins,
                                       reason="serialize colliding scatters")
                scat_insts[(e, cc)] = sc

            def emit_rs(h):
                cc_inst = nc.gpsimd.collective_compute(
                    "ReduceScatter",
                    mybir.AluOpType.add,
                    replica_groups=[list(range(N_CORES))],
                    ins=[y_acc.ap()[h * TH:(h + 1) * TH, :].opt()],
                    outs=[rs_b.ap()[h * rows:(h + 1) * rows, :].opt()],
                )
                for k in touch_q.get(h, ()):
                    if k in scat_insts:
                        add_dep_helper(cc_inst.ins, scat_insts[k].ins,
                                       reason="rs after scatters")
                for tq in range(TQ):
                    if (tq * QT < (h + 1) * TH) and ((tq + 1) * QT > h * TH):
                        add_dep_helper(cc_inst.ins, dense_wr[tq].ins,
                                       reason="rs after dense init")
                rs_insts[h] = cc_inst
                out_wr = nc.sync.dma_start(
                    y_out.ap()[h * rows:(h + 1) * rows, :],
                    rs_b.ap()[h * rows:(h + 1) * rows, :],
                )
                add_dep_helper(out_wr.ins, cc_inst.ins, reason="copy rs out")

            su_done = [False] * TQ

            def ensure_dense(tq):
                if dense_wr[tq] is None:
                    if not su_done[tq]:
                        emit_su(tq)
                        su_done[tq] = True
                    emit_dense(tq)

            def maybe_rs():
                for h in range(NHALF):
                    if rs_insts[h] is None:
                        if all(k in scat_insts for k in touch_q.get(h, ())):
                            for tq in range(TQ):
                                if (tq * QT < (h + 1) * TH) and ((tq + 1) * QT > h * TH):
                                    ensure_dense(tq)
                            emit_rs(h)
                        else:
                            break  # keep cc-stream order h ascending

            def scatter(e, cc):
                for tq in touch_tq.get((e, cc), ()):
                    ensure_dense(tq)
                emit_scatter(e, cc)

            # ---- emission order: shared front half first, then chunk-major
            # expert pipeline (gu blocks lazily), shared tail tiles on demand
            # when a scatter first touches them ----
            emit_su(0)
            su_done[0] = True
            emit_su(1)
            su_done[1] = True
            emit_dense(0)
            emit_dense(1)

            gu_blocks = 0
            for cc in range(NCC):
                while gu_blocks * 256 < (cc + 1) * 128:
                    a, b = segs[gu_blocks]
                    emit_gu(0, a, b)
                    emit_gu(1, a, b)
                    gu_blocks += 1
                emit_down(0, cc)
                emit_down(1, cc)
                scatter(0, cc)
                if cc > 0:
                    scatter(1, cc - 1)
                maybe_rs()
            scatter(1, NCC - 1)
            for tq in range(TQ):
                ensure_dense(tq)
            maybe_rs()
            assert all(r is not None for r in rs_insts)

    nc.compile()
    return nc


def _get_nc(key):
    if key not in _nc_cache:
        _nc_cache[key] = _build(*key)
    return _nc_cache[key]


def kernel(hidden_states, gate_w, expert_gate, expert_up, expert_down,
           shared_gate, shared_up, shared_down):
    global last_exec_time_ns
    B, S, Hh = hidden_states.shape
    x = np.asarray(hidden_states, np.float32).reshape(-1, Hh)

    # ---- host-side routing (the all-to-all dispatch, done as sharding) ----
    gw = np.asarray(gate_w, np.float32)
    logits = x @ gw.T
    scores = 1.0 / (1.0 + np.exp(-logits))
    # top-4 per token; stable sort matches jax.lax.top_k tie semantics
    order = np.argsort(-scores, axis=1, kind="stable")[:, :TOPK]
    topk_w = np.take_along_axis(scores, order, axis=1)
    topk_w = topk_w / (topk_w.sum(-1, keepdims=True) + 1e-20)
    comb = np.zeros((T, E), np.float32)
    np.add.at(comb, (np.arange(T)[:, None], order), topk_w)

    sel = np.zeros((T, E), dtype=bool)
    sel[np.arange(T)[:, None], order] = True
    counts = sel.sum(0)
    C_use = int(max(64, -(-int(counts.max()) // 64) * 64))
    C_use = min(C_use, T)
    C_pad = -(-C_use // 128) * 128
    NCC = C_pad // 128

    gidx_all = np.zeros((E, C_pad), np.int32)
    sidx_all = np.full((E, C_pad), OOB, np.int32)
    for e in range(E):
        lst = np.nonzero(sel[:, e])[0].astype(np.int32)
        gidx_all[e, :len(lst)] = lst
        sidx_all[e, :len(lst)] = lst

    # ---- cast / pack per-core inputs ----
    x16 = x.astype(np.float16)
    xT16 = np.ascontiguousarray(x16.T)
    eg = np.asarray(expert_gate, np.float32).astype(np.float16)
    eu = np.asarray(expert_up, np.float32).astype(np.float16)
    ed = np.asarray(expert_down, np.float32).astype(np.float16)
    sg = np.asarray(shared_gate, np.float32).astype(np.float16)
    su = np.asarray(shared_up, np.float32).astype(np.float16)
    sd = np.asarray(shared_down, np.float32).astype(np.float16)

    in_maps = []
    for c in range(N_CORES):
        ex = [EPC * c + k for k in range(EPC)]
        # gathered + transposed tokens per local expert: [e, o, p, C_pad]
        xgT = np.stack([
            np.ascontiguousarray(x16[gidx_all[e]].T.reshape(H // 128, 128, C_pad))
            for e in ex
        ])
        wgu = np.stack([
            np.stack([eg[e].reshape(H // 128, 128, I),
                      eu[e].reshape(H // 128, 128, I)])
            for e in ex
        ])
        wdp = np.stack([ed[e].reshape(I // 128, 128, H) for e in ex])
        wGc = np.stack([
            comb[gidx_all[e], e].astype(np.float32).reshape(NCC, 128)
            for e in ex
        ])
        # zero pad-row weights (pad rows also scatter to OOB and are dropped)
        for k, e in enumerate(ex):
            nreal = int(counts[e])
            wGc[k].reshape(-1)[nreal:] = 0.0
        in_maps.append({
            "xT16": xT16,
            "xgT16": xgT,
            "wgu16": np.ascontiguousarray(wgu),
            "wd16": np.ascontiguousarray(wdp),
            "sgsu16": np.ascontiguousarray(
                np.concatenate([sg[:, c * SIC:(c + 1) * SIC],
                                su[:, c * SIC:(c + 1) * SIC]], axis=1)),
            "sd16": np.ascontiguousarray(sd[c * SIC:(c + 1) * SIC, :]),
            "sidx": np.ascontiguousarray(sidx_all[ex].reshape(EPC, NCC, 128)),
            "wG": np.ascontiguousarray(wGc),
        })

    # collision edges between the two local experts' scatter chunks, plus the
    # dense tiles / RS chunks each (expert, chunk) scatter touches (union
    # across cores — SPMD shares one program)
    edge_set = set()
    touch_tq = {}
    touch_q = {}
    QT = T // 4
    for c in range(N_CORES):
        pair = [EPC * c, EPC * c + 1]
        rng = {}
        for k, e in enumerate(pair):
            for i in range(NCC):
                r = sidx_all[e, i * 128:(i + 1) * 128]
                r = r[r < OOB]
                if len(r):
                    lo, hi = int(r.min()), int(r.max())
                    rng[(k, i)] = (lo, hi)
                    for tq in range(lo // QT, hi // QT + 1):
                        touch_tq.setdefault((k, i), set()).add(tq)
                    for h in range(lo // TH, hi // TH + 1):
                        touch_q.setdefault(h, set()).add((k, i))
        for i in range(NCC):
            for jj in range(NCC):
                a = rng.get((0, i))
                b = rng.get((1, jj))
                if a and b and a[0] <= b[1] and b[0] <= a[1]:
                    edge_set.add((i, jj))
    edges = tuple(sorted(edge_set))
    touch_tq_t = tuple(sorted((k, tuple(sorted(v))) for k, v in touch_tq.items()))
    touch_q_t = tuple(sorted((h, tuple(sorted(v))) for h, v in touch_q.items()))

    key = (C_use, C_pad, edges, touch_tq_t, touch_q_t, NHALF)
    if key not in _nc_cache:
        _nc_cache[key] = _build(
            C_use, C_pad, edges,
            {k: v for k, v in touch_tq_t}, {h: v for h, v in touch_q_t})
    nc = _nc_cache[key]
    trace = bool(int(os.environ.get("KERNEL_TRACE", "0")))
    res = run_bass_kernel_spmd(
        nc, in_maps, core_ids=list(range(N_CORES)), trace=trace
    )
    last_exec_time_ns = res.exec_time_ns

    # reassemble: RS chunk h gives core c rows [h*TH + c*rows : +rows]
    rows = TH // N_CORES
    out = np.empty((T, Hh), np.float32)
    for c in range(N_CORES):
        yo = res.results[c]["y_out"]
        for h in range(NHALF):
            out[h * TH + c * rows:h * TH + (c + 1) * rows] = yo[h * rows:(h + 1) * rows]
    return out.reshape(B, S, Hh).astype(np.float32)


# revision 20
# speedup vs baseline: 1.1713x; 1.0327x over previous
"""DeepseekV3 MoE layer on 8 Trainium2 NeuronCores.

Strategy (expert-parallel, per sharding hint):
- Each core owns 2 of the 16 routed experts. The host routes tokens by top-4
  gate scores (fp32, identical to reference) and ships each core its experts'
  gathered tokens pre-transposed, plus the normalized combine weights
  (host-side gate math, same class of work as the top-k routing).
- All device inputs are shipped in their exact SBUF layouts so every preload
  DMA is contiguous per partition (8-16KB lines) and balanced across the
  three DMA queues (sync / scalar / gpsimd).
- The device runs the SwiGLU expert MLPs fp16 (fp32 PSUM), scales outputs by
  the combine weights into per-expert staging buffers, and scatter-adds them
  into a per-core partial-output y_acc in DRAM with two batched indirect
  DMAs per expert (low/high token halves) to keep the gpsimd engine free.
- The shared expert is sharded along its intermediate dim (128 of 1024 per
  core), computed weight-stationary so its intermediate lands pre-transposed
  ([i, t]); its dense per-tile output initializes y_acc.
- y_acc is reduce-scattered in NHALF token chunks, each fired as soon as the
  scatters touching that chunk complete, so all but the last chunk overlap
  compute. The host reassembles the fp16 outputs and casts (pure unshard).
"""

import os
import sys
import types

sys.path.insert(0, "/opt/trn_rl_repo")

# antenv.axon_hooks shim so trace=True works under axon (profiling only).
if "antenv.axon_hooks" not in sys.modules:
    _hook_holder = [None]
    _hooks_mod = types.ModuleType("antenv.axon_hooks")
    _hooks_mod.set_axon_ntff_profile_hook = lambda h: _hook_holder.__setitem__(0, h)
    _hooks_mod.get_axon_ntff_profile_hook = lambda: _hook_holder[0]
    sys.modules["antenv.axon_hooks"] = _hooks_mod
    try:
        from trn_agent_boot.trn_boot import _ntff_profile_via_ctypes

        _hook_holder[0] = _ntff_profile_via_ctypes("/opt/axon/libaxon_pjrt.so")
    except Exception:
        pass

import numpy as np

import concourse.bass as bass
import concourse.mybir as mybir
from concourse import bacc
from concourse.tile import TileContext, add_dep_helper
from concourse.bass_utils import run_bass_kernel_spmd

N_CORES = 8
T, H, E, I = 2048, 1024, 16, 512
TOPK = 4
SIC = 128  # shared-expert intermediate slice per core (1024 / 8)
EPC = 2  # experts per core
OOB = 1 << 20
NHALF = int(os.environ.get('KERNEL_NHALF', '2'))  # reduce-scatter chunks
TH = T // NHALF
TQ = 4  # dense-write granularity (512-token tiles)
QT = T // TQ

F16 = mybir.dt.float16
F32 = mybir.dt.float32
I32 = mybir.dt.int32
AF = mybir.ActivationFunctionType

_nc_cache = {}
last_exec_time_ns = None


def _build(C_use, C_pad, batches, edges, touch_tq, touch_q):
    """batches: tuple of (e, c0, c1) scatter batches in emission order.
    edges: chunk-level (i0, jj) pairs where expert-0 chunk i0 and expert-1
    chunk jj may touch the same y_acc rows (RMW adds must serialize).
    touch_tq[bi] / touch_q[h] are keyed by batch index."""
    NCC = C_pad // 128
    nc = bacc.Bacc(trn_type="TRN2", target_bir_lowering=False, num_devices=N_CORES)

    # ---- I/O (all pre-arranged to SBUF layout on host; contiguous DMAs) ----
    xT16 = nc.dram_tensor("xT16", [TQ, 128, H // 128, QT], F16, kind="ExternalInput")
    xgT16 = nc.dram_tensor("xgT16", [EPC, 128, H // 128, C_pad], F16, kind="ExternalInput")
    wgu16 = nc.dram_tensor("wgu16", [EPC, 128, 2, H // 128, I], F16, kind="ExternalInput")
    wd16 = nc.dram_tensor("wd16", [EPC, 128, I // 128, H], F16, kind="ExternalInput")
    sgsu16 = nc.dram_tensor("sgsu16", [128, H // 128, 2 * SIC], F16, kind="ExternalInput")
    sd16 = nc.dram_tensor("sd16", [SIC, H], F16, kind="ExternalInput")
    sidx = nc.dram_tensor("sidx", [128, EPC * NCC], I32, kind="ExternalInput")
    wG = nc.dram_tensor("wG", [128, EPC * NCC], F32, kind="ExternalInput")

    y_acc = nc.dram_tensor("y_acc", [T, H], F16)
    rows = TH // N_CORES
    rs_b = nc.dram_tensor("rs_b", [NHALF * rows, H], F16)
    y_out = nc.dram_tensor("y_out", [NHALF * rows, H], F16, kind="ExternalOutput")

    SS = 2 * SIC  # 256

    with TileContext(nc) as tc:
        with (
            tc.tile_pool(name="res", bufs=1) as res,
            tc.tile_pool(name="sc", bufs=4) as scp,
            tc.tile_pool(name="ds", bufs=2) as dsp,
            tc.tile_pool(name="ps_a", bufs=4, space="PSUM") as ps_a,
            tc.tile_pool(name="ps_gu", bufs=2, space="PSUM") as ps_gu,
        ):
            # ---- resident tiles ----
            xT_sb = [res.tile([128, H // 128, QT], F16, tag=f"xT{q}",
                              name=f"xT_sb{q}") for q in range(TQ)]
            xgT_sb = res.tile([128, EPC, H // 128, C_pad], F16, tag="xgT")
            wgu_sb = res.tile([128, EPC, 2, H // 128, I], F16, tag="wgu")
            wd_sb = res.tile([128, EPC, I // 128, H], F16, tag="wd")
            sgsu_sb = res.tile([128, H // 128, SS], F16, tag="sgsu")
            sd_sb = res.tile([128, H], F16, tag="sd")
            sidx_sb = res.tile([128, EPC * NCC], I32, tag="sidx")
            wG_sb = res.tile([128, EPC * NCC], F32, tag="wG")
            p_sb = res.tile([128, EPC, I // 128, C_pad], F16, tag="p")
            sp_sb = res.tile([128, T], F16, tag="sp")
            yg_sb = [res.tile([128, NCC, H], F16, tag=f"yg{e}",
                              name=f"yg_sb{e}") for e in range(EPC)]

            # ---- preload: balanced across the three DMA queues, each queue
            # ordered by first-use time ----
            nc.sync.dma_start(sd_sb[:], sd16.ap())
            for q in range(TQ):
                nc.sync.dma_start(xT_sb[q][:], xT16.ap()[q])
            nc.sync.dma_start(sidx_sb[:], sidx.ap())
            nc.sync.dma_start(wG_sb[:], wG.ap())

            nc.scalar.dma_start(sgsu_sb[:], sgsu16.ap())
            nc.scalar.dma_start(wgu_sb[:, 0], wgu16.ap()[0])
            for e in range(EPC):
                nc.scalar.dma_start(wd_sb[:, e], wd16.ap()[e])

            nc.gpsimd.dma_start(xgT_sb[:, 0], xgT16.ap()[0])
            nc.gpsimd.dma_start(wgu_sb[:, 1], wgu16.ap()[1])
            nc.gpsimd.dma_start(xgT_sb[:, 1], xgT16.ap()[1])

            # zero the pad columns of p (read by down-matmul lhsT chunks)
            if C_pad > C_use:
                nc.vector.memset(p_sb[:, :, :, C_use:C_pad], 0)

            # gate/up token blocks of 256 (ldweights still overlaps matmuls;
            # small blocks let downs/scatters/RS start early)
            segs = []
            s0 = 0
            while s0 < C_use:
                s1 = min(s0 + 256, C_use)
                segs.append((s0, s1))
                s0 = s1

            # ---- shared expert: weight-stationary gate/up; sp lands
            # pre-transposed [i, t] so the down matmul needs no transposes ----
            def emit_su(tq):
                ps_ic = []
                for ic in range(2):
                    psu = ps_a.tile([128, QT], F32, tag="psa")
                    for ho in range(H // 128):
                        nc.tensor.matmul(
                            psu[:],
                            lhsT=sgsu_sb[:, ho, ic * 128:(ic + 1) * 128],
                            rhs=xT_sb[tq][:, ho, :],
                            start=(ho == 0),
                            stop=(ho == H // 128 - 1),
                        )
                    ps_ic.append(psu)
                sg_t = scp.tile([128, QT], F16, tag="sg")
                nc.scalar.activation(sg_t[:], ps_ic[0][:], AF.Silu)
                nc.vector.tensor_tensor(
                    out=sp_sb[:, tq * QT:(tq + 1) * QT], in0=sg_t[:], in1=ps_ic[1][:],
                    op=mybir.AluOpType.mult,
                )

            dense_wr = [None] * TQ

            def emit_dense(tq):
                ys = dsp.tile([128, 4, H], F16, tag="ys")
                for tc4 in range(4):
                    t0 = tq * QT + tc4 * 128
                    for hf in range(2):
                        pso = ps_a.tile([128, 512], F32, tag="psa")
                        nc.tensor.matmul(
                            pso[:],
                            lhsT=sp_sb[:, t0:t0 + 128],
                            rhs=sd_sb[:, hf * 512:(hf + 1) * 512],
                            start=True,
                            stop=True,
                        )
                        nc.vector.tensor_copy(
                            ys[:, tc4, hf * 512:(hf + 1) * 512], pso[:])
                dense_wr[tq] = nc.scalar.dma_start(
                    y_acc.ap()[tq * QT:(tq + 1) * QT, :].rearrange(
                        "(tc p) h -> p tc h", p=128),
                    ys[:],
                )

            # ---- routed experts: g/u -> p = silu(g)*u for one token block ----
            def emit_gu(e, a, b):
                for it in range(I // 128):
                    pg_full = ps_gu.tile([128, 512], F32, tag="pg")
                    pg = pg_full[:, :b - a]
                    pu_full = ps_gu.tile([128, 512], F32, tag="pu")
                    pu = pu_full[:, :b - a]
                    for ho in range(H // 128):
                        nc.tensor.matmul(
                            pg[:],
                            lhsT=wgu_sb[:, e, 0, ho, it * 128:(it + 1) * 128],
                            rhs=xgT_sb[:, e, ho, a:b],
                            start=(ho == 0),
                            stop=(ho == H // 128 - 1),
                        )
                        nc.tensor.matmul(
                            pu[:],
                            lhsT=wgu_sb[:, e, 1, ho, it * 128:(it + 1) * 128],
                            rhs=xgT_sb[:, e, ho, a:b],
                            start=(ho == 0),
                            stop=(ho == H // 128 - 1),
                        )
                    sg2_full = scp.tile([128, 512], F16, tag="sg2")
                    sg2 = sg2_full[:, :b - a]
                    nc.scalar.activation(sg2[:], pg[:], AF.Silu)
                    nc.vector.tensor_tensor(
                        out=p_sb[:, e, it, a:b], in0=sg2[:], in1=pu[:],
                        op=mybir.AluOpType.mult,
                    )

            # ---- routed expert down matmul + combine-weight scale ----
            def emit_down(e, cc):
                j = e * NCC + cc
                for hf in range(2):
                    py = ps_a.tile([128, 512], F32, tag="psa")
                    for it in range(I // 128):
                        nc.tensor.matmul(
                            py[:],
                            lhsT=p_sb[:, e, it, cc * 128:(cc + 1) * 128],
                            rhs=wd_sb[:, e, it, hf * 512:(hf + 1) * 512],
                            start=(it == 0),
                            stop=(it == I // 128 - 1),
                        )
                    nc.vector.tensor_scalar_mul(
                        yg_sb[e][:, cc, hf * 512:(hf + 1) * 512],
                        py[:], wG_sb[:, j:j + 1])

            su_done = [False] * TQ

            def ensure_dense(tq):
                if dense_wr[tq] is None:
                    if not su_done[tq]:
                        emit_su(tq)
                        su_done[tq] = True
                    emit_dense(tq)

            scat_insts = {}  # (e, cc) -> inst
            batch_done = set()
            rs_insts = [None] * NHALF

            def emit_scatter(bi):
                e, c0, c1 = batches[bi]
                for tq in touch_tq.get(bi, ()):
                    ensure_dense(tq)
                for cc in range(c0, c1 + 1):
                    j = e * NCC + cc
                    sc = nc.gpsimd.indirect_dma_start(
                        out=y_acc[:],
                        out_offset=bass.IndirectOffsetOnAxis(
                            ap=sidx_sb[:, j:j + 1], axis=0),
                        in_=yg_sb[e][:, cc, :],
                        in_offset=None,
                        bounds_check=T - 1,
                        oob_is_err=False,
                        compute_op=mybir.AluOpType.add,
                    )
                    for tq in touch_tq.get(bi, ()):
                        add_dep_helper(sc.ins, dense_wr[tq].ins,
                                       reason="scatter after dense init")
                    for (i0, jj) in edges:
                        other = None
                        if e == 1 and jj == cc:
                            other = (0, i0)
                        elif e == 0 and i0 == cc:
                            other = (1, jj)
                        if other is not None and other in scat_insts:
                            add_dep_helper(sc.ins, scat_insts[other].ins,
                                           reason="serialize colliding scatters")
                    scat_insts[(e, cc)] = sc
                batch_done.add(bi)

            def maybe_rs():
                for h in range(NHALF):
                    if rs_insts[h] is None:
                        if all(bi in batch_done for bi in touch_q.get(h, ())):
                            for tq in range(TQ):
                                if (tq * QT < (h + 1) * TH) and ((tq + 1) * QT > h * TH):
                                    ensure_dense(tq)
                            cc_inst = nc.gpsimd.collective_compute(
                                "ReduceScatter",
                                mybir.AluOpType.add,
                                replica_groups=[list(range(N_CORES))],
                                ins=[y_acc.ap()[h * TH:(h + 1) * TH, :].opt()],
                                outs=[rs_b.ap()[h * rows:(h + 1) * rows, :].opt()],
                            )
                            for bi in touch_q.get(h, ()):
                                e, c0, c1 = batches[bi]
                                for cc in range(c0, c1 + 1):
                                    add_dep_helper(
                                        cc_inst.ins, scat_insts[(e, cc)].ins,
                                        reason="rs after scatters")
                            for tq in range(TQ):
                                if (tq * QT < (h + 1) * TH) and ((tq + 1) * QT > h * TH):
                                    add_dep_helper(cc_inst.ins, dense_wr[tq].ins,
                                                   reason="rs after dense init")
                            rs_insts[h] = cc_inst
                            out_wr = nc.sync.dma_start(
                                y_out.ap()[h * rows:(h + 1) * rows, :],
                                rs_b.ap()[h * rows:(h + 1) * rows, :],
                            )
                            add_dep_helper(out_wr.ins, cc_inst.ins,
                                           reason="copy rs out")
                        else:
                            break  # keep cc-stream order h ascending

            # ---- emission order: shared front tiles, then per-expert
            # low-half pipeline (so RS chunk 0 fires early), then high ----
            emit_su(0)
            su_done[0] = True
            emit_su(1)
            su_done[1] = True
            emit_dense(0)
            emit_dense(1)

            gu_blocks = [0, 0]

            def gu_through(e, cc):
                while gu_blocks[e] * 256 < (cc + 1) * 128:
                    a, b = segs[gu_blocks[e]]
                    emit_gu(e, a, b)
                    gu_blocks[e] += 1

            for bi, (e, c0, c1) in enumerate(batches):
                gu_through(e, c1)
                for cc in range(c0, c1 + 1):
                    emit_down(e, cc)
                emit_scatter(bi)
                maybe_rs()
            for tq in range(TQ):
                ensure_dense(tq)
            maybe_rs()
            assert all(r is not None for r in rs_insts)

    nc.compile()
    return nc


def kernel(hidden_states, gate_w, expert_gate, expert_up, expert_down,
           shared_gate, shared_up, shared_down):
    global last_exec_time_ns
    B, S, Hh = hidden_states.shape
    x = np.asarray(hidden_states, np.float32).reshape(-1, Hh)

    # ---- host-side routing (the all-to-all dispatch, done as sharding) ----
    gw = np.asarray(gate_w, np.float32)
    scores = 1.0 / (1.0 + np.exp(-(x @ gw.T)))
    order = np.argsort(-scores, axis=1, kind="stable")[:, :TOPK]
    topk_w = np.take_along_axis(scores, order, axis=1)
    topk_w = topk_w / (topk_w.sum(-1, keepdims=True) + 1e-20)
    comb = np.zeros((T, E), np.float32)
    np.add.at(comb, (np.arange(T)[:, None], order), topk_w)

    sel = np.zeros((T, E), dtype=bool)
    sel[np.arange(T)[:, None], order] = True
    counts = sel.sum(0)
    C_use = int(max(64, -(-int(counts.max()) // 64) * 64))
    C_use = min(C_use, T)
    C_pad = -(-C_use // 128) * 128
    NCC = C_pad // 128

    gidx_all = np.zeros((E, C_pad), np.int32)
    sidx_all = np.full((E, C_pad), OOB, np.int32)
    for e in range(E):
        lst = np.nonzero(sel[:, e])[0].astype(np.int32)
        gidx_all[e, :len(lst)] = lst
        sidx_all[e, :len(lst)] = lst

    # ---- scatter batches: split each expert's chunks at the token-half
    # boundary (uniform across cores for SPMD) ----
    # bc = last chunk whose min real token < TH, maxed over cores/experts
    bc = 0
    for e in range(E):
        for i in range(NCC):
            r = sidx_all[e, i * 128:(i + 1) * 128]
            r = r[r < OOB]
            if len(r) and int(r.min()) < TH:
                bc = max(bc, i)
    batches = [(0, 0, bc), (1, 0, bc)]
    if bc + 1 < NCC:
        batches += [(0, bc + 1, NCC - 1), (1, bc + 1, NCC - 1)]
    batches = tuple(batches)

    # ---- cast / pack per-core inputs in exact SBUF layouts ----
    x16 = x.astype(np.float16)
    xT4 = np.ascontiguousarray(
        x16.T.reshape(H // 128, 128, TQ, QT).transpose(2, 1, 0, 3))
    eg = np.asarray(expert_gate, np.float32).astype(np.float16)
    eu = np.asarray(expert_up, np.float32).astype(np.float16)
    ed = np.asarray(expert_down, np.float32).astype(np.float16)
    sg = np.asarray(shared_gate, np.float32).astype(np.float16)
    su = np.asarray(shared_up, np.float32).astype(np.float16)
    sd = np.asarray(shared_down, np.float32).astype(np.float16)

    in_maps = []
    for c in range(N_CORES):
        ex = [EPC * c + k for k in range(EPC)]
        xgT = np.stack([
            np.ascontiguousarray(
                x16[gidx_all[e]].T.reshape(H // 128, 128, C_pad).transpose(1, 0, 2))
            for e in ex
        ])
        wgu = np.stack([
            np.stack([eg[e], eu[e]]).reshape(2, H // 128, 128, I)
            .transpose(2, 0, 1, 3)
            for e in ex
        ])
        wdp = np.stack([
            ed[e].reshape(I // 128, 128, H).transpose(1, 0, 2) for e in ex
        ])
        wGc = np.stack([
            comb[gidx_all[e], e].astype(np.float32) for e in ex
        ])  # [EPC, C_pad]
        for k, e in enumerate(ex):
            wGc[k, int(counts[e]):] = 0.0
        in_maps.append({
            "xT16": xT4,
            "xgT16": xgT,
            "wgu16": np.ascontiguousarray(wgu),
            "wd16": np.ascontiguousarray(wdp),
            "sgsu16": np.ascontiguousarray(
                np.concatenate([sg[:, c * SIC:(c + 1) * SIC],
                                su[:, c * SIC:(c + 1) * SIC]], axis=1)
                .reshape(H // 128, 128, 2 * SIC).transpose(1, 0, 2)),
            "sd16": np.ascontiguousarray(sd[c * SIC:(c + 1) * SIC, :]),
            "sidx": np.ascontiguousarray(
                sidx_all[ex].reshape(EPC * NCC, 128).T),
            "wG": np.ascontiguousarray(wGc.reshape(EPC * NCC, 128).T),
        })

    # batch token ranges + chunk-level collision edges (union across cores —
    # SPMD shares one program)
    brange = {}
    edge_set = set()
    for c in range(N_CORES):
        rng = {}
        for k in range(EPC):
            e = EPC * c + k
            for i in range(NCC):
                r = sidx_all[e, i * 128:(i + 1) * 128]
                r = r[r < OOB]
                if len(r):
                    rng[(k, i)] = (int(r.min()), int(r.max()))
        for i in range(NCC):
            for jj in range(NCC):
                a = rng.get((0, i))
                b = rng.get((1, jj))
                if a and b and a[0] <= b[1] and b[0] <= a[1]:
                    edge_set.add((i, jj))
        for bi, (k, c0, c1) in enumerate(batches):
            e = EPC * c + k
            r = sidx_all[e, c0 * 128:(c1 + 1) * 128]
            r = r[r < OOB]
            if len(r):
                lo, hi = int(r.min()), int(r.max())
                old = brange.get(bi)
                brange[bi] = (min(old[0], lo), max(old[1], hi)) if old else (lo, hi)
    edges = tuple(sorted(edge_set))
    touch_tq = {}
    touch_q = {}
    for bi, (lo, hi) in brange.items():
        touch_tq[bi] = tuple(range(lo // QT, hi // QT + 1))
        for h in range(lo // TH, hi // TH + 1):
            touch_q.setdefault(h, set()).add(bi)
    touch_tq_t = tuple(sorted(touch_tq.items()))
    touch_q_t = tuple(sorted((h, tuple(sorted(v))) for h, v in touch_q.items()))

    key = (C_use, C_pad, batches, edges, touch_tq_t, touch_q_t, NHALF)
    if key not in _nc_cache:
        _nc_cache[key] = _build(
            C_use, C_pad, batches, edges,
            dict(touch_tq_t), {h: v for h, v in touch_q_t})
    nc = _nc_cache[key]
    trace = bool(int(os.environ.get("KERNEL_TRACE", "0")))
    res = run_bass_kernel_spmd(
        nc, in_maps, core_ids=list(range(N_CORES)), trace=trace
    )
    last_exec_time_ns = res.exec_time_ns

    # reassemble: RS chunk h gives core c rows [h*TH + c*rows : +rows]
    rows = TH // N_CORES
    out = np.empty((T, Hh), np.float32)
    for c in range(N_CORES):
        yo = res.results[c]["y_out"]
        for h in range(NHALF):
            out[h * TH + c * rows:h * TH + (c + 1) * rows] = yo[h * rows:(h + 1) * rows]
    return out.reshape(B, S, Hh).astype(np.float32)
